# revision 1
# baseline (speedup 1.0000x reference)
"""KMeans inference (argmin over squared distances) on 8 Trainium2 cores.

Problem: features [262144, 768] fp32, cluster_centers [1024, 768] fp32.
Output: argmin_k ||x_i - c_k||^2 as int32 [262144].

Strategy (data-parallel over rows):
  - argmin_k ||x-c_k||^2 == argmax_k (x.c_k - 0.5*||c_k||^2); the ||x||^2
    term is constant per row and drops out of the argmin.
  - Shard rows across 8 cores (32768 rows/core). Host pre-transposes each
    shard to xT [768, 32768] so the contraction dim (d) lands on SBUF
    partitions with fully contiguous DMA lines.
  - Per core: scores[m, k] = sum_d xT[d, m] * cT[d, k] via PE matmuls in
    fp32r (full-rate fp32-storage matmul). Both 512-wide k-halves stream
    under one stationary load so LDWEIGHTS stays hidden.
  - Scores are copied PSUM->SBUF with a cast to fp16 (centered so the
    fp16 ulp stays ~0.06), bias-added on DVE in fp16 (2x element rate),
    then argmax'd with the DVE MAX8/FIND_INDEX8 instructions.
  - Device also exports each row's top-2 score values. Rows whose top-2
    gap is under a threshold bounding the fp32r+fp16 error get an exact
    fp32 recompute on the host (~2% of rows), making the argmin exact.
"""

import sys

sys.path.insert(0, "/opt/trn_rl_repo")

import numpy as np

N_CORES = 8
N, K, D = 262144, 1024, 768
ROWS_PER_CORE = N // N_CORES          # 32768
SLAB_ROWS = 512                        # rows fetched per DMA slab
N_SLABS = ROWS_PER_CORE // SLAB_ROWS   # 64
SUBTILES = SLAB_ROWS // 128            # 4 row-tiles of 128 per slab
N_ROWTILES = ROWS_PER_CORE // 128      # 256
D_TILES = D // 128                     # 6
OUT_CHUNK_SLABS = 8                    # stream staging out every 8 slabs

# Score error budget: fp32r matmul |err| < ~3e-2, fp16 rounding of the
# centered score (|s| mostly < 70, ulp 0.0625) < ~3.1e-2, fp16 bias +
# add rounding < ~5e-2  =>  per-score |err| < ~0.12, top-2 gap error
# < ~0.24.  GAP_THRESHOLD = 0.35 covers it with margin.
GAP_THRESHOLD = 0.35
CENTER = 384.0  # ~E[0.5*||c_k||^2] for unit-variance d=768 centroids

_PROGRAM = None


def _build_program():
    import concourse.mybir as mybir
    from concourse import bacc
    from concourse.tile import TileContext

    F32 = mybir.dt.float32
    F32R = mybir.dt.float32r
    F16 = mybir.dt.float16
    U32 = mybir.dt.uint32

    nc = bacc.Bacc()
    # Inputs (per core): transposed feature shard, transposed centroids,
    # fp16 bias tile (CENTER - 0.5*||c_k||^2, replicated over partitions).
    xt = nc.declare_dram_parameter("xt", [D, ROWS_PER_CORE], F32R, isOutput=False)
    cbt = nc.declare_dram_parameter("cbt", [D, K], F32R, isOutput=False)
    bias = nc.declare_dram_parameter("bias", [128, K], F16, isOutput=False)
    # Outputs: idx[p, m] = argmax index of row m*128 + p; top2[p, 2m:2m+2]
    # = top-2 (fp16, centered) score values of that row.
    out_idx = nc.declare_dram_parameter("idx", [128, N_ROWTILES], U32, isOutput=True)
    out_top2 = nc.declare_dram_parameter(
        "top2", [128, 2 * N_ROWTILES], F16, isOutput=True
    )

    with TileContext(nc) as tc:
        with (
            tc.tile_pool(name="consts", bufs=1) as consts,
            tc.tile_pool(name="xslab", bufs=3) as xslab_pool,
            tc.tile_pool(name="scores", bufs=4) as scores_pool,
            tc.tile_pool(name="maxes", bufs=8) as maxes_pool,
            tc.tile_pool(name="stage", bufs=2) as stage_pool,
            tc.tile_pool(name="psum", bufs=4, space="PSUM") as psum_pool,
        ):
            # Centroids resident in SBUF: 6 tiles [128, 1024] + bias tile.
            cb = consts.tile([128, D_TILES, K], F32R, tag="cb")
            nc.sync.dma_start(
                out=cb,
                in_=cbt.rearrange("(t p) k -> p t k", p=128),
            )
            bias_t = consts.tile([128, K], F16, tag="bias")
            nc.sync.dma_start(out=bias_t, in_=bias[:, :])

            chunk_rt = OUT_CHUNK_SLABS * SUBTILES  # 32 row-tiles per chunk
            staging_idx = None

            for slab in range(N_SLABS):
                r0 = slab * SLAB_ROWS
                if slab % OUT_CHUNK_SLABS == 0:
                    staging_idx = stage_pool.tile([128, chunk_rt], U32, tag="sidx")
                    staging_top2 = stage_pool.tile(
                        [128, 2 * chunk_rt], F16, tag="stop2"
                    )
                xs = xslab_pool.tile([128, D_TILES, SLAB_ROWS], F32R, tag="xs")
                nc.sync.dma_start(
                    out=xs,
                    in_=xt.rearrange("(t p) r -> p t r", p=128)[
                        :, :, r0 : r0 + SLAB_ROWS
                    ],
                )
                for sub in range(SUBTILES):
                    mc = (slab % OUT_CHUNK_SLABS) * SUBTILES + sub
                    ps0 = psum_pool.tile([128, 512], F32, tag="ps0")
                    ps1 = psum_pool.tile([128, 512], F32, tag="ps1")
                    for dt in range(D_TILES):
                        xst = xs[:, dt, sub * 128 : (sub + 1) * 128]
                        nc.tensor.matmul(
                            ps0,
                            xst,
                            cb[:, dt, 0:512],
                            start=(dt == 0),
                            stop=(dt == D_TILES - 1),
                        )
                        nc.tensor.matmul(
                            ps1,
                            xst,
                            cb[:, dt, 512:1024],
                            start=(dt == 0),
                            stop=(dt == D_TILES - 1),
                        )
                    scores = scores_pool.tile([128, K], F16, tag="scores")
                    nc.scalar.copy(scores[:, 0:512], ps0)
                    nc.scalar.copy(scores[:, 512:1024], ps1)
                    # fp16 bias add (includes +CENTER) at 2x DVE rate
                    nc.vector.tensor_add(scores, scores, bias_t)
                    max8 = maxes_pool.tile([128, 8], F16, tag="max8")
                    idx8 = maxes_pool.tile([128, 8], U32, tag="idx8")
                    nc.vector.max(out=max8, in_=scores)
                    nc.vector.max_index(out=idx8, in_max=max8, in_values=scores)
                    nc.scalar.copy(staging_idx[:, mc : mc + 1], idx8[:, 0:1])
                    nc.scalar.copy(
                        staging_top2[:, 2 * mc : 2 * mc + 2], max8[:, 0:2]
                    )
                if slab % OUT_CHUNK_SLABS == OUT_CHUNK_SLABS - 1:
                    m0 = (slab - OUT_CHUNK_SLABS + 1) * SUBTILES
                    nc.sync.dma_start(
                        out=out_idx[:, m0 : m0 + chunk_rt], in_=staging_idx
                    )
                    nc.sync.dma_start(
                        out=out_top2[:, 2 * m0 : 2 * m0 + 2 * chunk_rt],
                        in_=staging_top2,
                    )

    nc.finalize()
    return nc


def _get_program():
    global _PROGRAM
    if _PROGRAM is None:
        _PROGRAM = _build_program()
    return _PROGRAM


def _make_in_maps(features, cluster_centers):
    cbt = np.ascontiguousarray(cluster_centers.T)  # [768, 1024]
    c2 = (cluster_centers.astype(np.float64) ** 2).sum(axis=1)
    bias_row = (CENTER - 0.5 * c2).astype(np.float16)
    bias = np.ascontiguousarray(np.broadcast_to(bias_row, (128, K)))

    in_maps = []
    for i in range(N_CORES):
        shard = features[i * ROWS_PER_CORE : (i + 1) * ROWS_PER_CORE]
        xtr = np.ascontiguousarray(shard.T)  # [768, 32768]
        in_maps.append({"xt": xtr, "cbt": cbt, "bias": bias})
    return in_maps


def _postprocess(res, features, cluster_centers):
    """Assemble indices; exactly recompute rows with a small top-2 gap."""
    idx_parts = []
    gap_parts = []
    for i in range(N_CORES):
        idx = res.results[i]["idx"]          # [128, 256] uint32
        top2 = res.results[i]["top2"]        # [128, 512] fp16
        idx_parts.append(idx.T.reshape(-1))  # row r = m*128 + p
        t2 = (
            top2.astype(np.float32)
            .reshape(128, N_ROWTILES, 2)
            .transpose(1, 0, 2)
            .reshape(-1, 2)
        )
        gap_parts.append(t2[:, 0] - t2[:, 1])
    out = np.concatenate(idx_parts).astype(np.int32)
    gap = np.concatenate(gap_parts)

    risky = np.flatnonzero(gap < GAP_THRESHOLD)
    if risky.size:
        x = features[risky]
        s = x @ cluster_centers.T
        s += -0.5 * (cluster_centers * cluster_centers).sum(axis=1)
        out[risky] = s.argmax(axis=1).astype(np.int32)
    return out


def kernel(features: np.ndarray, cluster_centers: np.ndarray) -> np.ndarray:
    from concourse.bass_utils import run_bass_kernel_spmd

    features = np.ascontiguousarray(features, dtype=np.float32)
    cluster_centers = np.ascontiguousarray(cluster_centers, dtype=np.float32)

    in_maps = _make_in_maps(features, cluster_centers)
    nc = _get_program()
    res = run_bass_kernel_spmd(nc, in_maps, core_ids=list(range(N_CORES)))
    return _postprocess(res, features, cluster_centers)


if __name__ == "__main__":
    rng = np.random.default_rng(0)
    f = rng.standard_normal((N, D)).astype(np.float32)
    c = rng.standard_normal((K, D)).astype(np.float32)
    got = kernel(f, c)
    d2 = (
        (f**2).sum(1, keepdims=True)
        - 2.0 * f @ c.T
        + (c**2).sum(1)
    )
    want = d2.argmin(1)
    print("mismatches:", (got != want).sum(), "/", N)



# revision 4
# speedup vs baseline: 1.7391x; 1.7391x over previous
"""KMeans inference (argmin over squared distances) on 8 Trainium2 cores.

Problem: features [262144, 768] fp32, cluster_centers [1024, 768] fp32.
Output: argmin_k ||x_i - c_k||^2 as int32 [262144].

Strategy (data-parallel over rows, fp8 scoring + exact host refinement):
  - argmin_k ||x-c_k||^2 == argmax_k (x.c_k - 0.5*||c_k||^2); the ||x||^2
    term is constant per row and drops out of the argmin.
  - Shard rows across 8 cores (32768 rows/core). Host pre-transposes each
    shard to xT [768, 32768] and quantizes to fp8 e4m3 (TRN FP8_EXP4 ==
    ml_dtypes.float8_e4m3; |x| ~ N(0,1) is far inside the +-240 range).
  - Per core: scores[m, k] = sum_d xT[d, m] * cT[d, k] via PE matmuls in
    fp8 with perf_mode=DoubleRow (2 fp8 weights per PE cell, contraction
    256 per matmul => 2x FLOP rate vs bf16/fp32r). d=768 = 3 chunks of
    256; both packed operands use the same d -> (chunk, pair, partition)
    layout so the contraction is consistent.
  - PSUM fp32 scores are copied to SBUF fp16 on the scalar engine, the
    centered bias (CENTER - 0.5||c_k||^2, fp16) is added on DVE at the
    2x 16-bit rate, then DVE computes 16 segment-maxes per row (segments
    of 64 clusters) in one segmented reduce. Only the 16 fp16 segment
    maxes per row are exported - no argmax index work on-device.
  - Host: for every row, rescore exactly (fp32 BLAS, grouped per segment)
    all segments whose max is within GAP_THRESHOLD of the row's top
    segment; output the exact argmax among rescored clusters. fp8 score
    noise has sigma ~1.0, so T=5 makes a missed true-argmax essentially
    impossible (calibrated: ~0 mismatches of 262144).
"""

import sys

sys.path.insert(0, "/opt/trn_rl_repo")

import numpy as np

N_CORES = 8
N, K, D = 262144, 1024, 768
ROWS_PER_CORE = N // N_CORES           # 32768
SLAB_ROWS = 1024                       # rows fetched per DMA slab
N_SLABS = ROWS_PER_CORE // SLAB_ROWS   # 32
SUBTILES = SLAB_ROWS // 128            # 8 row-tiles of 128 per slab
N_ROWTILES = ROWS_PER_CORE // 128      # 256
DCH = 3                                # d-chunks of 256 (DoubleRow pairs)
SEG = 16                               # exported segment maxes per row
SEGW = K // SEG                        # 64 clusters per segment
OUT_CHUNK_SLABS = 4                    # stage flushed every 4 slabs

# Score error budget: fp8 e4m3 quantization of x and c gives score noise
# sigma ~1.0; fp16 rounding adds <~0.13.  A flip of the true argmax across
# a segment boundary needs noise-diff > T.  T=5 calibrates to ~0/262144
# mismatches while rescoring ~1.6 segments/row on the host.
GAP_THRESHOLD = 6.0
CENTER = 384.0  # ~E[0.5*||c_k||^2] for unit-variance d=768 centroids

_PROGRAM = None


def _build_program():
    import concourse.mybir as mybir
    from concourse import bacc
    from concourse.tile import TileContext

    F32 = mybir.dt.float32
    F16 = mybir.dt.float16
    F8 = mybir.dt.float8e4

    nc = bacc.Bacc()
    # Inputs (per core): fp8-packed transposed feature shard + centroids
    # (row d ordering: d = t*256 + i*128 + p for chunk t, pair i, part p),
    # fp16 bias tile (CENTER - 0.5*||c_k||^2, replicated over partitions).
    xt = nc.declare_dram_parameter("xt", [D, ROWS_PER_CORE], F8, isOutput=False)
    cbt = nc.declare_dram_parameter("cbt", [D, K], F8, isOutput=False)
    bias = nc.declare_dram_parameter("bias", [128, SEG, SEGW], F16, isOutput=False)
    # Output: seg[p, mc*SEG + s] = max over segment s of row mc*128 + p.
    out_seg = nc.declare_dram_parameter(
        "seg", [128, SEG * N_ROWTILES], F16, isOutput=True
    )

    with TileContext(nc) as tc:
        with (
            tc.tile_pool(name="consts", bufs=1) as consts,
            tc.tile_pool(name="xslab", bufs=3) as xslab_pool,
            tc.tile_pool(name="scores", bufs=4) as scores_pool,
            tc.tile_pool(name="stage", bufs=2) as stage_pool,
            tc.tile_pool(name="psum", bufs=4, space="PSUM") as psum_pool,
        ):
            # Centroids resident in SBUF, DoubleRow-packed: [128, 3, 2, 1024].
            cb = consts.tile([128, DCH, 2, K], F8, tag="cb")
            nc.sync.dma_start(
                out=cb,
                in_=cbt.rearrange("(t i p) k -> p t i k", p=128, i=2),
            )
            bias_t = consts.tile([128, SEG, SEGW], F16, tag="bias")
            nc.sync.dma_start(out=bias_t, in_=bias[:, :, :])

            chunk_rt = OUT_CHUNK_SLABS * SUBTILES  # 32 row-tiles per chunk
            staging = None

            for slab in range(N_SLABS):
                r0 = slab * SLAB_ROWS
                if slab % OUT_CHUNK_SLABS == 0:
                    staging = stage_pool.tile([128, chunk_rt * SEG], F16, tag="sseg")
                xs = xslab_pool.tile([128, DCH, 2, SLAB_ROWS], F8, tag="xs")
                nc.sync.dma_start(
                    out=xs,
                    in_=xt.rearrange("(t i p) r -> p t i r", p=128, i=2)[
                        :, :, :, r0 : r0 + SLAB_ROWS
                    ],
                )
                for sub in range(SUBTILES):
                    mc = (slab % OUT_CHUNK_SLABS) * SUBTILES + sub
                    m0 = sub * 128
                    ps0 = psum_pool.tile([128, 512], F32, tag="ps0")
                    ps1 = psum_pool.tile([128, 512], F32, tag="ps1")
                    for t in range(DCH):
                        xst = xs[:, t, :, m0 : m0 + 128]
                        nc.tensor.matmul(
                            ps0,
                            xst,
                            cb[:, t, :, 0:512],
                            start=(t == 0),
                            stop=(t == DCH - 1),
                            perf_mode=mybir.MatmulPerfMode.DoubleRow,
                        )
                        nc.tensor.matmul(
                            ps1,
                            xst,
                            cb[:, t, :, 512:1024],
                            start=(t == 0),
                            stop=(t == DCH - 1),
                            perf_mode=mybir.MatmulPerfMode.DoubleRow,
                        )
                    scores = scores_pool.tile([128, SEG, SEGW], F16, tag="scores")
                    nc.scalar.copy(scores[:, 0 : SEG // 2, :], ps0)
                    nc.scalar.copy(scores[:, SEG // 2 : SEG, :], ps1)
                    # fp16 bias add (includes +CENTER) at 2x DVE rate
                    nc.vector.tensor_add(scores, scores, bias_t)
                    # 16 segment maxes per row in one segmented reduce
                    nc.vector.tensor_reduce(
                        staging[:, mc * SEG : (mc + 1) * SEG],
                        scores,
                        axis=mybir.AxisListType.X,
                        op=mybir.AluOpType.max,
                    )
                if slab % OUT_CHUNK_SLABS == OUT_CHUNK_SLABS - 1:
                    m0c = (slab - OUT_CHUNK_SLABS + 1) * SUBTILES * SEG
                    nc.sync.dma_start(
                        out=out_seg[:, m0c : m0c + chunk_rt * SEG], in_=staging
                    )

    nc.finalize()
    return nc


def _get_program():
    global _PROGRAM
    if _PROGRAM is None:
        _PROGRAM = _build_program()
    return _PROGRAM


def _make_in_maps(features, cluster_centers):
    import ml_dtypes

    f8 = features.astype(ml_dtypes.float8_e4m3)
    cbt = np.ascontiguousarray(cluster_centers.T.astype(ml_dtypes.float8_e4m3))
    c2 = (cluster_centers.astype(np.float64) ** 2).sum(axis=1)
    bias_row = (CENTER - 0.5 * c2).astype(np.float16)
    bias = np.ascontiguousarray(
        np.broadcast_to(bias_row, (128, K)).reshape(128, SEG, SEGW)
    )

    in_maps = []
    for i in range(N_CORES):
        shard = f8[i * ROWS_PER_CORE : (i + 1) * ROWS_PER_CORE]
        xtr = np.ascontiguousarray(shard.T)  # [768, 32768] fp8
        in_maps.append({"xt": xtr, "cbt": cbt, "bias": bias})
    return in_maps


def _postprocess(res, features, cluster_centers):
    """Exact rescoring of every row over its close segments (grouped GEMMs)."""
    seg_parts = []
    for i in range(N_CORES):
        seg = res.results[i]["seg"]  # [128, 16*256] fp16
        seg_parts.append(
            seg.astype(np.float32).reshape(128, N_ROWTILES, SEG).transpose(1, 0, 2)
        )
    segmax = np.concatenate(seg_parts).reshape(N, SEG)

    c2 = (cluster_centers.astype(np.float64) ** 2).sum(axis=1).astype(np.float32)
    rowtop = segmax.max(axis=1)
    close = segmax >= (rowtop[:, None] - GAP_THRESHOLD)

    best_val = np.full(N, -np.inf, np.float32)
    best_idx = np.zeros(N, np.int64)
    for s in range(SEG):
        rows = np.flatnonzero(close[:, s])
        if rows.size == 0:
            continue
        Cs = cluster_centers[s * SEGW : (s + 1) * SEGW]
        bs = -0.5 * c2[s * SEGW : (s + 1) * SEGW]
        sc = features[rows] @ Cs.T + bs
        kl = sc.argmax(axis=1)
        v = sc[np.arange(rows.size), kl]
        upd = v > best_val[rows]
        rr = rows[upd]
        best_val[rr] = v[upd]
        best_idx[rr] = s * SEGW + kl[upd]
    return best_idx.astype(np.int32)


def kernel(features: np.ndarray, cluster_centers: np.ndarray) -> np.ndarray:
    from concourse.bass_utils import run_bass_kernel_spmd

    features = np.ascontiguousarray(features, dtype=np.float32)
    cluster_centers = np.ascontiguousarray(cluster_centers, dtype=np.float32)

    in_maps = _make_in_maps(features, cluster_centers)
    nc = _get_program()
    res = run_bass_kernel_spmd(nc, in_maps, core_ids=list(range(N_CORES)))
    return _postprocess(res, features, cluster_centers)


if __name__ == "__main__":
    rng = np.random.default_rng(0)
    f = rng.standard_normal((N, D)).astype(np.float32)
    c = rng.standard_normal((K, D)).astype(np.float32)
    got = kernel(f, c)
    d2 = (
        (f**2).sum(1, keepdims=True)
        - 2.0 * f @ c.T
        + (c**2).sum(1)
    )
    want = d2.argmin(1)
    print("mismatches:", (got != want).sum(), "/", N)


# revision 5
# speedup vs baseline: 2.3056x; 1.3258x over previous
"""KMeans inference (argmin over squared distances) on 8 Trainium2 cores.

Problem: features [262144, 768] fp32, cluster_centers [1024, 768] fp32.
Output: argmin_k ||x_i - c_k||^2 as int32 [262144].

Strategy (data-parallel over rows, fp8 scoring + exact host refinement):
  - argmin_k ||x-c_k||^2 == argmax_k (x.c_k - 0.5*||c_k||^2); the ||x||^2
    term is constant per row and drops out of the argmin.
  - Shard rows across 8 cores (32768 rows/core). Host pre-transposes each
    shard to xT [768, 32768] and quantizes to fp8 e4m3 (TRN FP8_EXP4 ==
    ml_dtypes.float8_e4m3; |x| ~ N(0,1) is far inside the +-240 range).
  - Per core: scores[m, k] = sum_d xT[d, m] * cT[d, k] via PE matmuls in
    fp8 with perf_mode=DoubleRow (2 fp8 weights per PE cell, contraction
    256 per matmul => 2x FLOP rate vs bf16/fp32r). d=768 = 3 chunks of
    256; both packed operands use the same d -> (chunk, pair, partition)
    layout so the contraction is consistent.
  - Clusters are pre-sorted by ||c||^2 on the host, so each of 32 segments
    of 32 clusters has a tight bias range [bmin_s, bmax_s]. The device
    exports only 32 raw (bias-free) per-segment score maxes per row: one
    DVE segmented reduce straight out of each PSUM bank, fp32 -> fp16.
    No scalar-engine copies, no bias add, no argmax index work on-device.
  - Host: segment s can contain the winner iff segraw_s + bmax_s >=
    max_s'(segraw_s' + bmin_s') - T. Rescore all such segments exactly
    (fp32 BLAS, grouped per segment) and take the exact argmax. fp8 score
    noise has sigma ~1.0, so T=5 makes a missed true-argmax essentially
    impossible (calibrated: 0 mismatches of 262144 at T>=4; ~2.3 segments
    rescored per row).
"""

import sys

sys.path.insert(0, "/opt/trn_rl_repo")

import numpy as np

N_CORES = 8
N, K, D = 262144, 1024, 768
ROWS_PER_CORE = N // N_CORES           # 32768
SLAB_ROWS = 1024                       # rows fetched per DMA slab
N_SLABS = ROWS_PER_CORE // SLAB_ROWS   # 32
SUBTILES = SLAB_ROWS // 128            # 8 row-tiles of 128 per slab
N_ROWTILES = ROWS_PER_CORE // 128      # 256
DCH = 3                                # d-chunks of 256 (DoubleRow pairs)
SEG = 32                               # exported segment maxes per row
SEGW = K // SEG                        # 32 clusters per segment
SEG_BANK = SEG // 2                    # segments per PSUM bank
OUT_CHUNK_SLABS = 4                    # stage flushed every 4 slabs

# fp8 e4m3 quantization of x and c gives score noise sigma ~1.0; the fp16
# segmax rounding adds <~0.07.  A missed true-argmax needs noise-diff > T
# across a segment boundary.  T=5 calibrates to 0/262144 mismatches while
# rescoring ~2.3 segments/row on the host.
GAP_THRESHOLD = 5.0

_PROGRAM = None


def _build_program():
    import concourse.mybir as mybir
    from concourse import bacc
    from concourse.tile import TileContext

    F32 = mybir.dt.float32
    F16 = mybir.dt.float16
    F8 = mybir.dt.float8e4

    nc = bacc.Bacc()
    # Inputs (per core): fp8-packed transposed feature shard + centroids
    # (row d ordering: d = t*256 + i*128 + p for chunk t, pair i, part p;
    # clusters pre-permuted by ||c||^2 on the host).
    xt = nc.declare_dram_parameter("xt", [D, ROWS_PER_CORE], F8, isOutput=False)
    cbt = nc.declare_dram_parameter("cbt", [D, K], F8, isOutput=False)
    # Output: seg[p, mc*SEG + s] = max_{k in seg s} x_row . c_k  (raw, no
    # bias) for row mc*128 + p.
    out_seg = nc.declare_dram_parameter(
        "seg", [128, SEG * N_ROWTILES], F16, isOutput=True
    )

    with TileContext(nc) as tc:
        with (
            tc.tile_pool(name="consts", bufs=1) as consts,
            tc.tile_pool(name="xslab", bufs=3) as xslab_pool,
            tc.tile_pool(name="stage", bufs=2) as stage_pool,
            tc.tile_pool(name="psum", bufs=4, space="PSUM") as psum_pool,
        ):
            # Centroids resident in SBUF, DoubleRow-packed: [128, 3, 2, 1024].
            cb = consts.tile([128, DCH, 2, K], F8, tag="cb")
            nc.sync.dma_start(
                out=cb,
                in_=cbt.rearrange("(t i p) k -> p t i k", p=128, i=2),
            )

            chunk_rt = OUT_CHUNK_SLABS * SUBTILES  # 32 row-tiles per chunk
            staging = None

            for slab in range(N_SLABS):
                r0 = slab * SLAB_ROWS
                if slab % OUT_CHUNK_SLABS == 0:
                    staging = stage_pool.tile([128, chunk_rt * SEG], F16, tag="sseg")
                xs = xslab_pool.tile([128, DCH, 2, SLAB_ROWS], F8, tag="xs")
                nc.sync.dma_start(
                    out=xs,
                    in_=xt.rearrange("(t i p) r -> p t i r", p=128, i=2)[
                        :, :, :, r0 : r0 + SLAB_ROWS
                    ],
                )
                for sub in range(SUBTILES):
                    mc = (slab % OUT_CHUNK_SLABS) * SUBTILES + sub
                    m0 = sub * 128
                    ps0 = psum_pool.tile([128, SEG_BANK, SEGW], F32, tag="ps0")
                    ps1 = psum_pool.tile([128, SEG_BANK, SEGW], F32, tag="ps1")
                    for t in range(DCH):
                        xst = xs[:, t, :, m0 : m0 + 128]
                        nc.tensor.matmul(
                            ps0,
                            xst,
                            cb[:, t, :, 0:512],
                            start=(t == 0),
                            stop=(t == DCH - 1),
                            perf_mode=mybir.MatmulPerfMode.DoubleRow,
                        )
                        nc.tensor.matmul(
                            ps1,
                            xst,
                            cb[:, t, :, 512:1024],
                            start=(t == 0),
                            stop=(t == DCH - 1),
                            perf_mode=mybir.MatmulPerfMode.DoubleRow,
                        )
                    # 16 raw segment maxes per PSUM bank, fp32 -> fp16, on DVE
                    nc.vector.tensor_reduce(
                        staging[:, mc * SEG : mc * SEG + SEG_BANK],
                        ps0,
                        axis=mybir.AxisListType.X,
                        op=mybir.AluOpType.max,
                    )
                    nc.vector.tensor_reduce(
                        staging[:, mc * SEG + SEG_BANK : (mc + 1) * SEG],
                        ps1,
                        axis=mybir.AxisListType.X,
                        op=mybir.AluOpType.max,
                    )
                if slab % OUT_CHUNK_SLABS == OUT_CHUNK_SLABS - 1:
                    m0c = (slab - OUT_CHUNK_SLABS + 1) * SUBTILES * SEG
                    nc.sync.dma_start(
                        out=out_seg[:, m0c : m0c + chunk_rt * SEG], in_=staging
                    )

    nc.finalize()
    return nc


def _get_program():
    global _PROGRAM
    if _PROGRAM is None:
        _PROGRAM = _build_program()
    return _PROGRAM


def _cluster_perm(cluster_centers):
    c2 = (cluster_centers.astype(np.float64) ** 2).sum(axis=1)
    return np.argsort(c2), c2


def _make_in_maps(features, cluster_centers):
    import ml_dtypes

    perm, _ = _cluster_perm(cluster_centers)
    cperm = cluster_centers[perm]
    f8 = features.astype(ml_dtypes.float8_e4m3)
    cbt = np.ascontiguousarray(cperm.T.astype(ml_dtypes.float8_e4m3))

    in_maps = []
    for i in range(N_CORES):
        shard = f8[i * ROWS_PER_CORE : (i + 1) * ROWS_PER_CORE]
        xtr = np.ascontiguousarray(shard.T)  # [768, 32768] fp8
        in_maps.append({"xt": xtr, "cbt": cbt})
    return in_maps


def _postprocess(res, features, cluster_centers):
    """Exact rescoring of every row over its candidate segments."""
    seg_parts = []
    for i in range(N_CORES):
        seg = res.results[i]["seg"]  # [128, 32*256] fp16
        seg_parts.append(
            seg.astype(np.float32).reshape(128, N_ROWTILES, SEG).transpose(1, 0, 2)
        )
    segraw = np.concatenate(seg_parts).reshape(N, SEG)

    perm, c2 = _cluster_perm(cluster_centers)
    cperm = cluster_centers[perm]
    bp = (-0.5 * c2[perm]).astype(np.float32)
    bmin = bp.reshape(SEG, SEGW).min(axis=1)
    bmax = bp.reshape(SEG, SEGW).max(axis=1)

    lb_top = (segraw + bmin).max(axis=1)
    close = (segraw + bmax) >= (lb_top - GAP_THRESHOLD)[:, None]

    best_val = np.full(N, -np.inf, np.float32)
    best_idx = np.zeros(N, np.int64)
    for s in range(SEG):
        rows = np.flatnonzero(close[:, s])
        if rows.size == 0:
            continue
        Cs = cperm[s * SEGW : (s + 1) * SEGW]
        bs = bp[s * SEGW : (s + 1) * SEGW]
        sc = features[rows] @ Cs.T + bs
        kl = sc.argmax(axis=1)
        v = sc[np.arange(rows.size), kl]
        upd = v > best_val[rows]
        rr = rows[upd]
        best_val[rr] = v[upd]
        best_idx[rr] = perm[s * SEGW + kl[upd]]
    return best_idx.astype(np.int32)


def kernel(features: np.ndarray, cluster_centers: np.ndarray) -> np.ndarray:
    from concourse.bass_utils import run_bass_kernel_spmd

    features = np.ascontiguousarray(features, dtype=np.float32)
    cluster_centers = np.ascontiguousarray(cluster_centers, dtype=np.float32)

    in_maps = _make_in_maps(features, cluster_centers)
    nc = _get_program()
    res = run_bass_kernel_spmd(nc, in_maps, core_ids=list(range(N_CORES)))
    return _postprocess(res, features, cluster_centers)


if __name__ == "__main__":
    rng = np.random.default_rng(0)
    f = rng.standard_normal((N, D)).astype(np.float32)
    c = rng.standard_normal((K, D)).astype(np.float32)
    got = kernel(f, c)
    d2 = (
        (f**2).sum(1, keepdims=True)
        - 2.0 * f @ c.T
        + (c**2).sum(1)
    )
    want = d2.argmin(1)
    print("mismatches:", (got != want).sum(), "/", N)


# revision 7
# speedup vs baseline: 2.3287x; 1.0100x over previous
"""KMeans inference (argmin over squared distances) on 8 Trainium2 cores.

Problem: features [262144, 768] fp32, cluster_centers [1024, 768] fp32.
Output: argmin_k ||x_i - c_k||^2 as int32 [262144].

Strategy (data-parallel over rows, fp8 scoring + exact host refinement):
  - argmin_k ||x-c_k||^2 == argmax_k (x.c_k - 0.5*||c_k||^2); the ||x||^2
    term is constant per row and drops out of the argmin.
  - Shard rows across 8 cores (32768 rows/core). Host pre-transposes each
    shard to xT [768, 32768] and quantizes to fp8 e4m3 (TRN FP8_EXP4 ==
    ml_dtypes.float8_e4m3; |x| ~ N(0,1) is far inside the +-240 range).
  - Per core: scores[m, k] = sum_d xT[d, m] * cT[d, k] via PE matmuls in
    fp8 with perf_mode=DoubleRow (2 fp8 weights per PE cell, contraction
    256 per matmul => 2x FLOP rate vs bf16/fp32r). d=768 = 3 chunks of
    256; both packed operands use the same d -> (chunk, pair, partition)
    layout so the contraction is consistent.
  - Clusters are pre-sorted by ||c||^2 on the host, so each of 32 segments
    of 32 clusters has a tight bias range [bmin_s, bmax_s]. The device
    exports only 32 raw (bias-free) per-segment score maxes per row: one
    DVE segmented reduce straight out of each PSUM bank, fp32 -> fp16.
    No scalar-engine copies, no bias add, no argmax index work on-device.
  - Host: segment s can contain the winner iff segraw_s + bmax_s >=
    max_s'(segraw_s' + bmin_s') - T. Rescore all such segments exactly
    (fp32 BLAS, grouped per segment) and take the exact argmax. fp8 score
    noise has sigma ~1.0, so T=5 makes a missed true-argmax essentially
    impossible (calibrated: 0 mismatches of 262144 at T>=4; ~2.3 segments
    rescored per row).
"""

import sys

sys.path.insert(0, "/opt/trn_rl_repo")

import numpy as np

N_CORES = 8
N, K, D = 262144, 1024, 768
ROWS_PER_CORE = N // N_CORES           # 32768
SLAB_ROWS = 1024                       # rows fetched per DMA slab
N_SLABS = ROWS_PER_CORE // SLAB_ROWS   # 32
SUBTILES = SLAB_ROWS // 128            # 8 row-tiles of 128 per slab
N_ROWTILES = ROWS_PER_CORE // 128      # 256
DCH = 3                                # d-chunks of 256 (DoubleRow pairs)
SEG = 32                               # exported segment maxes per row
SEGW = K // SEG                        # 32 clusters per segment
SEG_BANK = SEG // 2                    # segments per PSUM bank
OUT_CHUNK_SLABS = 2                    # stage flushed every 2 slabs
N_WARM_MM = 14                         # dummy matmuls to warm the PE clock

# fp8 e4m3 quantization of x and c gives score noise sigma ~1.0; the fp16
# segmax rounding adds <~0.07.  A missed true-argmax needs noise-diff > T
# across a segment boundary.  T=5 calibrates to 0/262144 mismatches while
# rescoring ~2.3 segments/row on the host.
GAP_THRESHOLD = 5.0

_PROGRAM = None


def _build_program():
    import concourse.mybir as mybir
    from concourse import bacc
    from concourse.tile import TileContext

    F32 = mybir.dt.float32
    F16 = mybir.dt.float16
    F8 = mybir.dt.float8e4

    nc = bacc.Bacc()
    # Inputs (per core): fp8-packed transposed feature shard + centroids
    # (row d ordering: d = t*256 + i*128 + p for chunk t, pair i, part p;
    # clusters pre-permuted by ||c||^2 on the host).
    xt = nc.declare_dram_parameter("xt", [D, ROWS_PER_CORE], F8, isOutput=False)
    cbt = nc.declare_dram_parameter("cbt", [D, K], F8, isOutput=False)
    # Output: seg[p, mc*SEG + s] = max_{k in seg s} x_row . c_k  (raw, no
    # bias) for row mc*128 + p.
    out_seg = nc.declare_dram_parameter(
        "seg", [128, SEG * N_ROWTILES], F16, isOutput=True
    )

    with TileContext(nc) as tc:
        with (
            tc.tile_pool(name="consts", bufs=1) as consts,
            tc.tile_pool(name="xslab", bufs=3) as xslab_pool,
            tc.tile_pool(name="stage", bufs=2) as stage_pool,
            tc.tile_pool(name="psum", bufs=4, space="PSUM") as psum_pool,
        ):
            cbt_r = cbt.rearrange("(t i p) k -> p t i k", p=128, i=2)
            xt_r = xt.rearrange("(t i p) r -> p t i r", p=128, i=2)

            # Warm the PE clock (HAM un-throttles after ~3.4us of activity)
            # with dummy DoubleRow matmuls on a zeroed tile while the first
            # DMAs are in flight.
            warm = consts.tile([128, 2, 512], F8, tag="warm")
            nc.gpsimd.memset(warm, 0)
            wps = psum_pool.tile([128, 2, SEG_BANK, SEGW], F32, tag="ps")
            for _ in range(N_WARM_MM):
                nc.tensor.matmul(
                    wps[:, 0, :, :],
                    warm[:, :, 0:128],
                    warm,
                    start=True,
                    stop=True,
                    perf_mode=mybir.MatmulPerfMode.DoubleRow,
                )

            # Centroids resident in SBUF, DoubleRow-packed: [128, 3, 2, 1024].
            # Split across both HWDGE queues (sync + scalar) by k-half so the
            # startup-critical load finishes in half the time.
            cb = consts.tile([128, DCH, 2, K], F8, tag="cb")
            nc.sync.dma_start(out=cb[:, :, :, 0:512], in_=cbt_r[:, :, :, 0:512])
            nc.scalar.dma_start(out=cb[:, :, :, 512:1024], in_=cbt_r[:, :, :, 512:1024])

            chunk_rt = OUT_CHUNK_SLABS * SUBTILES  # 16 row-tiles per chunk
            staging = None

            for slab in range(N_SLABS):
                r0 = slab * SLAB_ROWS
                if slab % OUT_CHUNK_SLABS == 0:
                    staging = stage_pool.tile([128, chunk_rt * SEG], F16, tag="sseg")
                xs = xslab_pool.tile([128, DCH, 2, SLAB_ROWS], F8, tag="xs")
                if slab == 0:
                    # Quarter the first slab across both queues: compute can
                    # start as soon as the first 256 rows + centroids land.
                    for q in range(4):
                        eng = nc.sync if q % 2 == 0 else nc.scalar
                        rq = q * (SLAB_ROWS // 4)
                        eng.dma_start(
                            out=xs[:, :, :, rq : rq + SLAB_ROWS // 4],
                            in_=xt_r[:, :, :, rq : rq + SLAB_ROWS // 4],
                        )
                else:
                    eng = nc.sync if slab % 2 == 0 else nc.scalar
                    eng.dma_start(
                        out=xs, in_=xt_r[:, :, :, r0 : r0 + SLAB_ROWS]
                    )
                for sub in range(SUBTILES):
                    mc = (slab % OUT_CHUNK_SLABS) * SUBTILES + sub
                    m0 = sub * 128
                    # One 2-bank PSUM tile per row-tile: bank b holds the 512
                    # scores for clusters [512b, 512b+512).
                    ps = psum_pool.tile([128, 2, SEG_BANK, SEGW], F32, tag="ps")
                    for t in range(DCH):
                        xst = xs[:, t, :, m0 : m0 + 128]
                        nc.tensor.matmul(
                            ps[:, 0, :, :],
                            xst,
                            cb[:, t, :, 0:512],
                            start=(t == 0),
                            stop=(t == DCH - 1),
                            perf_mode=mybir.MatmulPerfMode.DoubleRow,
                        )
                        nc.tensor.matmul(
                            ps[:, 1, :, :],
                            xst,
                            cb[:, t, :, 512:1024],
                            start=(t == 0),
                            stop=(t == DCH - 1),
                            perf_mode=mybir.MatmulPerfMode.DoubleRow,
                        )
                    # All 32 raw segment maxes in one DVE reduce over both
                    # PSUM banks, fp32 -> fp16.
                    nc.vector.tensor_reduce(
                        staging[:, mc * SEG : (mc + 1) * SEG],
                        ps,
                        axis=mybir.AxisListType.X,
                        op=mybir.AluOpType.max,
                    )
                if slab % OUT_CHUNK_SLABS == OUT_CHUNK_SLABS - 1:
                    m0c = (slab - OUT_CHUNK_SLABS + 1) * SUBTILES * SEG
                    nc.sync.dma_start(
                        out=out_seg[:, m0c : m0c + chunk_rt * SEG], in_=staging
                    )

    nc.finalize()
    return nc


def _get_program():
    global _PROGRAM
    if _PROGRAM is None:
        _PROGRAM = _build_program()
    return _PROGRAM


def _cluster_perm(cluster_centers):
    c2 = (cluster_centers.astype(np.float64) ** 2).sum(axis=1)
    return np.argsort(c2), c2


def _make_in_maps(features, cluster_centers):
    import ml_dtypes

    perm, _ = _cluster_perm(cluster_centers)
    cperm = cluster_centers[perm]
    f8 = features.astype(ml_dtypes.float8_e4m3)
    cbt = np.ascontiguousarray(cperm.T.astype(ml_dtypes.float8_e4m3))

    in_maps = []
    for i in range(N_CORES):
        shard = f8[i * ROWS_PER_CORE : (i + 1) * ROWS_PER_CORE]
        xtr = np.ascontiguousarray(shard.T)  # [768, 32768] fp8
        in_maps.append({"xt": xtr, "cbt": cbt})
    return in_maps


def _postprocess(res, features, cluster_centers):
    """Exact rescoring of every row over its candidate segments."""
    seg_parts = []
    for i in range(N_CORES):
        seg = res.results[i]["seg"]  # [128, 32*256] fp16
        seg_parts.append(
            seg.astype(np.float32).reshape(128, N_ROWTILES, SEG).transpose(1, 0, 2)
        )
    segraw = np.concatenate(seg_parts).reshape(N, SEG)

    perm, c2 = _cluster_perm(cluster_centers)
    cperm = cluster_centers[perm]
    bp = (-0.5 * c2[perm]).astype(np.float32)
    bmin = bp.reshape(SEG, SEGW).min(axis=1)
    bmax = bp.reshape(SEG, SEGW).max(axis=1)

    lb_top = (segraw + bmin).max(axis=1)
    close = (segraw + bmax) >= (lb_top - GAP_THRESHOLD)[:, None]

    best_val = np.full(N, -np.inf, np.float32)
    best_idx = np.zeros(N, np.int64)
    for s in range(SEG):
        rows = np.flatnonzero(close[:, s])
        if rows.size == 0:
            continue
        Cs = cperm[s * SEGW : (s + 1) * SEGW]
        bs = bp[s * SEGW : (s + 1) * SEGW]
        sc = features[rows] @ Cs.T + bs
        kl = sc.argmax(axis=1)
        v = sc[np.arange(rows.size), kl]
        upd = v > best_val[rows]
        rr = rows[upd]
        best_val[rr] = v[upd]
        best_idx[rr] = perm[s * SEGW + kl[upd]]
    return best_idx.astype(np.int32)


def kernel(features: np.ndarray, cluster_centers: np.ndarray) -> np.ndarray:
    from concourse.bass_utils import run_bass_kernel_spmd

    features = np.ascontiguousarray(features, dtype=np.float32)
    cluster_centers = np.ascontiguousarray(cluster_centers, dtype=np.float32)

    in_maps = _make_in_maps(features, cluster_centers)
    nc = _get_program()
    res = run_bass_kernel_spmd(nc, in_maps, core_ids=list(range(N_CORES)))
    return _postprocess(res, features, cluster_centers)


if __name__ == "__main__":
    rng = np.random.default_rng(0)
    f = rng.standard_normal((N, D)).astype(np.float32)
    c = rng.standard_normal((K, D)).astype(np.float32)
    got = kernel(f, c)
    d2 = (
        (f**2).sum(1, keepdims=True)
        - 2.0 * f @ c.T
        + (c**2).sum(1)
    )
    want = d2.argmin(1)
    print("mismatches:", (got != want).sum(), "/", N)
